# revision 1
# baseline (speedup 1.0000x reference)
"""Multi-head attention (B=2, S=2048, D=1024, H=16, E=64) on 8 NeuronCores.

Sharding: core c = (batch b, head-group hg) with b = c // 4, hg = c % 4.
Each core projects q/k/v for its batch into its 4 heads, runs dense
attention for those heads over the full sequence, and computes a partial
output projection with its 256 rows of Wo.  The host sums the 4 partials
per batch and adds bo (the TP all-reduce, folded into the gather step).

On-chip layout (everything "T" = feature-on-partitions):
  qhT/khT  [256, 2048]  two SBUF tiles [128, 2048]; head h at rows (h%2)*64
  vh'      [2048, 260]  16 tiles-worth in one [128, 4160] tile; per head a
                        65-wide block [vh | ones-col] - the ones column makes
                        the PV matmul emit the softmax denominator as row 64.
  scoresT  psum [t=128, s-pair 1024]; exp evicted by ACT with scale=1/8.
  biases are folded into the projection matmuls via a K=1 matmul against a
  ones row (weights staged host-side with the bias as row 1024).
"""

import numpy as np

B, S, D, H, E = 2, 2048, 1024, 16, 64
HG = 4            # heads per core
N_CORES = 8
EL = E + 1        # 65: head block width in vh' (values + ones column)
DT = D // 128     # 8 contraction tiles
SC = S // 512     # 4 s-chunks of 512

_NC = None        # cached compiled Bass module

# E_pair: 8 blocks [16, 128]; block (j, sc) broadcasts recip row (2j+m//64)*4+sc
# to output partition m — builds the per-head recip tile for a head-pair column
_EALL = np.zeros((16, 16 * E), np.float32)
for _j in range(2):
    for _sc in range(4):
        for _m in range(128):
            _EALL[(2 * _j + _m // 64) * 4 + _sc, (_j * 4 + _sc) * 128 + _m] = 1.0
_ONES = np.ones((1, 512), np.float32)


def _build():
    import concourse.bass as bass
    import concourse.mybir as mybir
    import concourse.tile as tile
    from concourse import bacc

    FP = mybir.dt.float32
    FPR = mybir.dt.float32r
    BF = mybir.dt.bfloat16
    EXP = mybir.ActivationFunctionType.Exp

    nc = bacc.Bacc("TRN2", target_bir_lowering=False, debug=False, num_devices=1)

    xq = nc.dram_tensor("xq", [D, S], BF, kind="ExternalInput").ap()
    xk = nc.dram_tensor("xk", [D, S], BF, kind="ExternalInput").ap()
    xv = nc.dram_tensor("xv", [D, S], BF, kind="ExternalInput").ap()
    wq = nc.dram_tensor("wq", [D + 1, HG * E], BF, kind="ExternalInput").ap()
    wk = nc.dram_tensor("wk", [D + 1, HG * E], BF, kind="ExternalInput").ap()
    wv = nc.dram_tensor("wv", [D + 1, HG * EL], BF, kind="ExternalInput").ap()
    wo = nc.dram_tensor("wo", [HG * E, D], FPR, kind="ExternalInput").ap()
    eall_d = nc.dram_tensor("eall", [16, 16 * E], FPR, kind="ExternalInput").ap()
    ones_d = nc.dram_tensor("ones", [1, 512], BF, kind="ExternalInput").ap()
    out = nc.dram_tensor("out_partial", [S, D], FP, kind="ExternalOutput").ap()

    with tile.TileContext(nc) as tc:
        with (
            tc.tile_pool(name="consts", bufs=1) as cpool,
            tc.tile_pool(name="resident", bufs=1) as rpool,
            tc.tile_pool(name="xin", bufs=12) as xpool,
            tc.tile_pool(name="exp", bufs=3) as epool,
            tc.tile_pool(name="outev", bufs=4) as opool,
            tc.tile_pool(name="stage", bufs=4) as spool,
        ):
            ones = cpool.tile([1, 512], BF, tag="ones")
            nc.gpsimd.dma_start(ones[:], ones_d[:])

            # E_all[k, r*64+j] = (k == r): broadcasts recip row r via matmul
            e_all = cpool.tile([16, 16 * E], FPR, tag="eall")
            nc.gpsimd.dma_start(e_all[:], eall_d[:])

            wq_sb = cpool.tile([128, DT * 256], BF, tag="wq")
            wk_sb = cpool.tile([128, DT * 256], BF, tag="wk")
            wv_sb = cpool.tile([128, DT * 260], BF, tag="wv")
            wqb = cpool.tile([1, 256], BF, tag="wqb")
            wkb = cpool.tile([1, 256], BF, tag="wkb")
            wvb = cpool.tile([1, 260], BF, tag="wvb")
            for dt in range(DT):
                nc.gpsimd.dma_start(
                    wq_sb[:, dt * 256 : (dt + 1) * 256],
                    wq[dt * 128 : (dt + 1) * 128, :],
                )
                nc.gpsimd.dma_start(
                    wk_sb[:, dt * 256 : (dt + 1) * 256],
                    wk[dt * 128 : (dt + 1) * 128, :],
                )
                nc.gpsimd.dma_start(
                    wv_sb[:, dt * 260 : (dt + 1) * 260],
                    wv[dt * 128 : (dt + 1) * 128, :],
                )
            nc.gpsimd.dma_start(wqb[:], wq[D : D + 1, :])
            nc.gpsimd.dma_start(wkb[:], wk[D : D + 1, :])
            nc.gpsimd.dma_start(wvb[:], wv[D : D + 1, :])

            wo_sb = []
            for j in range(2):
                t = cpool.tile([128, D], FPR, tag=f"wo{j}")
                nc.gpsimd.dma_start(t[:], wo[j * 128 : (j + 1) * 128, :])
                wo_sb.append(t)

            qhT = rpool.tile([128, 2 * S], FPR, tag="qhT")
            khT = rpool.tile([128, 2 * S], FPR, tag="khT")
            vh = rpool.tile([128, 16 * 260], BF, tag="vh")
            attnT = rpool.tile([128, 2 * S], FPR, tag="attnT")
            sums = rpool.tile([16, 512], FPR, tag="sums")
            recip = rpool.tile([16, 512], FPR, tag="recip")

            # ---- phase 1: projections ------------------------------------
            # q/k: dt-outer with all 8 (j, sc) psums open, so one weight
            # load (lhsT) serves 4 s-chunk matmuls instead of reloading
            # the stationary operand every matmul.
            with tc.tile_pool(name="ps_proj", bufs=8, space="PSUM") as pp:
                for x_dram, w_sb, w_b, dst in (
                    (xq, wq_sb, wqb, qhT),
                    (xk, wk_sb, wkb, khT),
                ):
                    pss = {}
                    for j in range(2):
                        for sc in range(SC):
                            pss[j, sc] = pp.tile(
                                [128, 512], FP, tag="pp", name=f"pp_{j}_{sc}"
                            )
                    xt = {}
                    for dt in range(DT):
                        for sc in range(SC):
                            t = xpool.tile([128, 512], BF, tag="xin")
                            nc.sync.dma_start(
                                t[:],
                                x_dram[dt * 128 : (dt + 1) * 128, sc * 512 : (sc + 1) * 512],
                            )
                            xt[sc] = t
                        for j in range(2):
                            for sc in range(SC):
                                nc.tensor.matmul(
                                    pss[j, sc][:],
                                    w_sb[:, dt * 256 + j * 128 : dt * 256 + (j + 1) * 128],
                                    xt[sc][:],
                                    start=(dt == 0),
                                    stop=False,
                                )
                    for j in range(2):
                        for sc in range(SC):
                            nc.tensor.matmul(
                                pss[j, sc][:],
                                w_b[0:1, j * 128 : (j + 1) * 128],
                                ones[0:1, :],
                                start=False,
                                stop=True,
                            )
                            nc.vector.tensor_copy(
                                dst[:, j * S + sc * 512 : j * S + (sc + 1) * 512],
                                pss[j, sc][:],
                            )
            # v: vh' tiles [t=128, 260] per 128-key block
            with tc.tile_pool(name="ps_vproj", bufs=2, space="PSUM") as pv:
                for sc in range(SC):
                    xt = []
                    for dt in range(DT):
                        t = xpool.tile([128, 512], BF, tag="xin")
                        nc.sync.dma_start(
                            t[:],
                            xv[dt * 128 : (dt + 1) * 128, sc * 512 : (sc + 1) * 512],
                        )
                        xt.append(t)
                    for u in range(4):
                        tt = sc * 4 + u
                        ps = pv.tile([128, 260], FP, tag="ppv")
                        for dt in range(DT):
                            nc.tensor.matmul(
                                ps[:],
                                xt[dt][:, u * 128 : (u + 1) * 128],
                                wv_sb[:, dt * 260 : (dt + 1) * 260],
                                start=(dt == 0),
                                stop=False,
                            )
                        nc.tensor.matmul(
                            ps[:],
                            ones[0:1, 0:128],
                            wvb[0:1, :],
                            start=False,
                            stop=True,
                        )
                        nc.vector.tensor_copy(
                            vh[:, tt * 260 : (tt + 1) * 260], ps[:]
                        )

            # ---- phase 2: attention --------------------------------------
            with (
                tc.tile_pool(name="ps_sc", bufs=2, space="PSUM") as psc,
                tc.tile_pool(name="ps_pv", bufs=2, space="PSUM") as ppv,
            ):
                for hp in range(2):          # head pair
                    h0, h1 = 2 * hp, 2 * hp + 1
                    for sc in range(SC):     # 512 queries
                        pv0 = ppv.tile([EL, 512], FP, tag="pv0")
                        pv1 = ppv.tile([EL, 512], FP, tag="pv1")
                        exq = []

                        def scores(tt):
                            ps = psc.tile([128, 1024], FP, tag="sc")
                            nc.tensor.matmul(
                                ps[:, 0:512],
                                khT[0:64, hp * S + tt * 128 : hp * S + (tt + 1) * 128],
                                qhT[0:64, hp * S + sc * 512 : hp * S + (sc + 1) * 512],
                                start=True,
                                stop=True,
                            )
                            nc.tensor.matmul(
                                ps[:, 512:1024],
                                khT[64:128, hp * S + tt * 128 : hp * S + (tt + 1) * 128],
                                qhT[64:128, hp * S + sc * 512 : hp * S + (sc + 1) * 512],
                                start=True,
                                stop=True,
                            )
                            ex = epool.tile([128, 1024], BF, tag="exp")
                            nc.scalar.activation(ex[:], ps[:], EXP, scale=0.125)
                            exq.append(ex)

                        def pv(tt):
                            ex = exq[tt]
                            nc.tensor.matmul(
                                pv0[:],
                                vh[:, tt * 260 + (h0 % 4) * EL : tt * 260 + (h0 % 4) * EL + EL],
                                ex[:, 0:512],
                                start=(tt == 0),
                                stop=(tt == 15),
                            )
                            nc.tensor.matmul(
                                pv1[:],
                                vh[:, tt * 260 + (h1 % 4) * EL : tt * 260 + (h1 % 4) * EL + EL],
                                ex[:, 512:1024],
                                start=(tt == 0),
                                stop=(tt == 15),
                            )

                        scores(0)
                        for tt in range(1, 16):
                            scores(tt)
                            pv(tt - 1)
                        pv(15)

                        r0, r1 = h0 * SC + sc, h1 * SC + sc
                        st0 = spool.tile([EL, 512], FPR, tag="stage")
                        st1 = spool.tile([EL, 512], FPR, tag="stage")
                        nc.vector.tensor_copy(st0[:], pv0[:])
                        nc.vector.tensor_copy(st1[:], pv1[:])
                        nc.gpsimd.dma_start(
                            attnT[0:64, hp * S + sc * 512 : hp * S + (sc + 1) * 512],
                            st0[0:E, :],
                        )
                        nc.gpsimd.dma_start(
                            attnT[64:128, hp * S + sc * 512 : hp * S + (sc + 1) * 512],
                            st1[0:E, :],
                        )
                        nc.gpsimd.dma_start(sums[r0 : r0 + 1, :], st0[E : E + 1, :])
                        nc.gpsimd.dma_start(sums[r1 : r1 + 1, :], st1[E : E + 1, :])

            # ---- phase 3: normalize + output projection ------------------
            with nc.allow_low_precision(reason="fp32r recip feeds fp32r matmul"):
                nc.vector.reciprocal(recip[:], sums[:])
            with (
                tc.tile_pool(name="ps_rb", bufs=2, space="PSUM") as prb,
                tc.tile_pool(name="ps_op", bufs=4, space="PSUM") as pop,
            ):
                for j in range(2):
                    for sc in range(SC):
                        rb = prb.tile([128, 512], FP, tag="rb")
                        nc.tensor.matmul(
                            rb[:],
                            e_all[:, (j * 4 + sc) * 128 : (j * 4 + sc + 1) * 128],
                            recip[:],
                            start=True,
                            stop=True,
                        )
                        sl = attnT[:, j * S + sc * 512 : j * S + (sc + 1) * 512]
                        nc.vector.tensor_mul(sl, sl, rb[:])
                for st in range(16):
                    for nh in range(2):
                        ps = pop.tile([128, 512], FP, tag="op")
                        for j in range(2):
                            nc.tensor.matmul(
                                ps[:],
                                attnT[:, j * S + st * 128 : j * S + (st + 1) * 128],
                                wo_sb[j][:, nh * 512 : (nh + 1) * 512],
                                start=(j == 0),
                                stop=(j == 1),
                            )
                        ot = opool.tile([128, 512], FP, tag="outev")
                        if nh == 0:
                            nc.vector.tensor_copy(ot[:], ps[:])
                        else:
                            nc.scalar.copy(ot[:], ps[:])
                        nc.sync.dma_start(
                            out[st * 128 : (st + 1) * 128, nh * 512 : (nh + 1) * 512],
                            ot[:],
                        )

    nc.compile()
    return nc


def _get_nc():
    global _NC
    if _NC is None:
        _NC = _build()
    return _NC


def _in_maps(q, k, v, Wq, bq, Wk, bk, Wv, bv, Wo, bo):
    import ml_dtypes
    f32 = np.float32
    bf16 = ml_dtypes.bfloat16
    maps = []
    for c in range(N_CORES):
        b, hg = c // HG, c % HG
        hs = slice(hg * HG, (hg + 1) * HG)  # this core's 4 heads

        wq_h = np.zeros((D + 1, HG * E), f32)
        wq_h[:D] = np.transpose(Wq[hs], (1, 0, 2)).reshape(D, HG * E)
        wq_h[D] = bq[hs].reshape(-1)
        wk_h = np.zeros((D + 1, HG * E), f32)
        wk_h[:D] = np.transpose(Wk[hs], (1, 0, 2)).reshape(D, HG * E)
        wk_h[D] = bk[hs].reshape(-1)
        wv_h = np.zeros((D + 1, HG * EL), f32)
        for hl in range(HG):
            wv_h[:D, hl * EL : hl * EL + E] = Wv[hg * HG + hl]
            wv_h[D, hl * EL : hl * EL + E] = bv[hg * HG + hl]
            wv_h[D, hl * EL + E] = 1.0  # generates the ones column of vh'
        maps.append(
            {
                "xq": np.ascontiguousarray(q[b].T).astype(bf16),
                "xk": np.ascontiguousarray(k[b].T).astype(bf16),
                "xv": np.ascontiguousarray(v[b].T).astype(bf16),
                "wq": wq_h.astype(bf16),
                "wk": wk_h.astype(bf16),
                "wv": wv_h.astype(bf16),
                "wo": np.ascontiguousarray(
                    Wo[hg * HG * E : (hg + 1) * HG * E, :], dtype=f32
                ),
                "eall": _EALL,
                "ones": _ONES.astype(bf16),
            }
        )
    return maps


def _run(inputs, trace=False):
    from concourse.bass_utils import run_bass_kernel_spmd

    nc = _get_nc()
    maps = _in_maps(**inputs)
    res = run_bass_kernel_spmd(nc, maps, list(range(N_CORES)), trace=trace)
    bo = np.asarray(inputs["bo"], np.float32)
    out = np.zeros((B, S, D), np.float32)
    for b in range(B):
        acc = np.zeros((S, D), np.float32)
        for hg in range(HG):
            acc += res.results[b * HG + hg]["out_partial"]
        out[b] = acc + bo[None, :]
    return out, res.exec_time_ns


def kernel(**inputs):
    out, _ = _run(inputs, trace=False)
    return out


def kernel_traced(**inputs):
    return _run(inputs, trace=True)



# revision 8
# speedup vs baseline: 1.0203x; 1.0203x over previous
"""Multi-head attention (B=2, S=2048, D=1024, H=16, E=64) on 8 NeuronCores.

Sharding: core c = (batch b, head-group hg) with b = c // 4, hg = c % 4.
Each core projects q/k/v for its batch into its 4 heads, runs dense
attention over the full sequence, and computes a partial output
projection with its 256 rows of Wo.  The host sums the 4 partials per
batch and adds bo (with the v-bias contribution bv @ Wo folded in).

V2 structure (vs the 350us baseline):
  - scores psum is evicted to SBUF fp16, so EXP runs as 32 ACTs of
    [128, 4096] from SBUF instead of 128 ACTs of [128, 1024] from
    PSUM: ~20% less scalar-engine time and no psum backpressure.
  - per-head processing: scores/EXP for head h interleave with the
    PV accumulation of head h-1 (and with the v projection for h=0),
    so the tensor engine never sits behind the scalar engine.
  - PV accumulates a whole head into one [65, 2048] psum (16 chained
    matmuls, 2048-col moving operand); the 65th row is the softmax
    denominator via a ones-column in vh (memset, not a bias matmul).
  - q/k/v biases: q/k folded into the psum eviction (tensor_scalar_add
    with a per-partition bias vector); v bias folded host-side into bo
    (bv @ Wo).  No K=1 bias matmuls.
  - projections stream x tiles once (dt-outer, all 4 (j, sc2) psums
    open) with 1024-col moving operands.
  - attnT / recip stay fp32r (denominator precision); q/k/attn inputs
    bf16; out f32.
"""

import numpy as np

B, S, D, H, E = 2, 2048, 1024, 16, 64
HG = 4            # heads per core
N_CORES = 8
EL = E + 1        # 65: head block width in vh (values + ones column)
DT = D // 128     # 8 contraction tiles

_NC = None        # cached compiled Bass module


def _build():
    import concourse.bass as bass
    import concourse.mybir as mybir
    import concourse.tile as tile
    from concourse import bacc

    FP = mybir.dt.float32
    FPR = mybir.dt.float32r
    BF = mybir.dt.bfloat16
    F16 = mybir.dt.float16
    EXP = mybir.ActivationFunctionType.Exp

    nc = bacc.Bacc("TRN2", target_bir_lowering=False, debug=False, num_devices=1)

    xq = nc.dram_tensor("xq", [D, S], BF, kind="ExternalInput").ap()
    xk = nc.dram_tensor("xk", [D, S], BF, kind="ExternalInput").ap()
    xv = nc.dram_tensor("xv", [D, S], BF, kind="ExternalInput").ap()
    wq = nc.dram_tensor("wq", [D, HG * E], BF, kind="ExternalInput").ap()
    wk = nc.dram_tensor("wk", [D, HG * E], BF, kind="ExternalInput").ap()
    wv = nc.dram_tensor("wv", [D, HG * E], BF, kind="ExternalInput").ap()
    qkb = nc.dram_tensor("qkb", [128, 4], FP, kind="ExternalInput").ap()
    wo = nc.dram_tensor("wo", [HG * E, D], FPR, kind="ExternalInput").ap()
    eall_d = nc.dram_tensor("eall", [HG, 2, 128], FPR, kind="ExternalInput").ap()
    out = nc.dram_tensor("out_partial", [S, D], FP, kind="ExternalOutput").ap()

    with tile.TileContext(nc) as tc:
        with (
            tc.tile_pool(name="consts", bufs=1) as cpool,
            tc.tile_pool(name="resident", bufs=1) as rpool,
            tc.tile_pool(name="xqk", bufs=3) as xqkp,
            tc.tile_pool(name="xvin", bufs=2) as xvp,
            tc.tile_pool(name="scsb", bufs=3) as scsb,
            tc.tile_pool(name="exbuf", bufs=4) as expool,
            tc.tile_pool(name="stage", bufs=2) as stp,
            tc.tile_pool(name="outev", bufs=2) as osb,
        ):
            wq_sb = cpool.tile([128, DT, 256], BF, tag="wq")
            wk_sb = cpool.tile([128, DT, 256], BF, tag="wk")
            wv_sb = cpool.tile([128, DT, 256], BF, tag="wv")
            qkb_sb = cpool.tile([128, 4], FP, tag="qkb")
            eall = cpool.tile([HG, 2, 128], FPR, tag="eall")
            wo_sb = []
            for j in range(2):
                t = cpool.tile([128, D], FPR, tag=f"wo{j}")
                nc.gpsimd.dma_start(t[:], wo[j * 128 : (j + 1) * 128, :])
                wo_sb.append(t)
            for dt in range(DT):
                nc.gpsimd.dma_start(wq_sb[:, dt, :], wq[dt * 128 : (dt + 1) * 128, :])
                nc.gpsimd.dma_start(wk_sb[:, dt, :], wk[dt * 128 : (dt + 1) * 128, :])
                nc.gpsimd.dma_start(wv_sb[:, dt, :], wv[dt * 128 : (dt + 1) * 128, :])
            nc.gpsimd.dma_start(qkb_sb[:], qkb[:])
            nc.gpsimd.dma_start(eall[:], eall_d[:])

            qhT = rpool.tile([128, 2, S], BF, tag="qhT")
            khT = rpool.tile([128, 2, S], BF, tag="khT")
            vh = rpool.tile([128, 16, HG, EL], BF, tag="vh")
            attnT = rpool.tile([128, 2, S], FPR, tag="attnT")
            sums = rpool.tile([HG, S], FPR, tag="sums")
            recip = rpool.tile([HG, S], FPR, tag="recip")

            # ones column of vh (softmax denominator accumulates in the
            # PV matmul); projection evictions only write cols 0:64.
            nc.vector.memset(vh[:, :, :, E : E + 1], 1.0)

            # ---- phase 1: q/k projections --------------------------------
            # dt-outer with all 4 (j, sc2) psums open: each x tile is
            # DMA'd once and read by 4 matmuls; w stationary serves 2.
            with tc.tile_pool(name="ps_proj", bufs=4, space="PSUM") as pp:
                for x_dram, w_sb, bcol, dst in (
                    (xq, wq_sb, 0, qhT),
                    (xk, wk_sb, 2, khT),
                ):
                    pss = {}
                    for j in range(2):
                        for sc2 in range(2):
                            pss[j, sc2] = pp.tile(
                                [128, 1024], FP, tag="pp", name=f"pp_{bcol}_{j}_{sc2}"
                            )
                    for dt in range(DT):
                        xt = xqkp.tile([128, S], BF, tag="xqk")
                        nc.sync.dma_start(xt[:], x_dram[dt * 128 : (dt + 1) * 128, :])
                        for j in range(2):
                            for sc2 in range(2):
                                for hc in range(2):
                                    nc.tensor.matmul(
                                        pss[j, sc2][:, hc * 512 : (hc + 1) * 512],
                                        w_sb[:, dt, j * 128 : (j + 1) * 128],
                                        xt[:, sc2 * 1024 + hc * 512 : sc2 * 1024 + (hc + 1) * 512],
                                        start=(dt == 0),
                                        stop=(dt == DT - 1),
                                    )
                    for j in range(2):
                        for sc2 in range(2):
                            nc.vector.tensor_scalar_add(
                                dst[:, j, sc2 * 1024 : (sc2 + 1) * 1024],
                                pss[j, sc2][:],
                                qkb_sb[:, bcol + j : bcol + j + 1],
                            )

            # ---- phase 2: attention (+ v projection interleaved) ---------
            exq_prev = {}   # ex tiles of the previous head, consumed by PV
            exq_cur = {}
            pvt = [None]    # open PV psum of the previous head
            cur_sc = [None]

            def scores_unit(h, tt, scp):
                hp, hr = h // 2, (h % 2) * 64
                p, tl = tt // 2, tt % 2
                if tl == 0:
                    cur_sc[0] = scsb.tile([128, 4096], F16, tag="scsb", name=f"sc_{h}_{tt}")
                for sc2 in range(2):
                    ps = scp.tile([128, 1024], FP, tag="scps", name=f"scps_{h}_{tt}_{sc2}")
                    for hc in range(2):
                        nc.tensor.matmul(
                            ps[:, hc * 512 : (hc + 1) * 512],
                            khT[hr : hr + 64, hp, tt * 128 : (tt + 1) * 128],
                            qhT[hr : hr + 64, hp, sc2 * 1024 + hc * 512 : sc2 * 1024 + (hc + 1) * 512],
                            start=True,
                            stop=True,
                        )
                    nc.vector.tensor_copy(
                        cur_sc[0][:, tl * 2048 + sc2 * 1024 : tl * 2048 + (sc2 + 1) * 1024],
                        ps[:],
                    )
                if tl == 1:
                    exb = expool.tile([128, 4096], BF, tag="exb", name=f"exb_{h}_{p}")
                    nc.scalar.activation(exb[:], cur_sc[0][:], EXP, scale=0.125)
                    exq_cur[p] = exb

            def pv_pair(h, p, pvp):
                if p == 0:
                    pvt[0] = pvp.tile([EL, S], FP, tag="pv", name=f"pv_{h}")
                for tl in range(2):
                    for qc in range(4):
                        nc.tensor.matmul(
                            pvt[0][:, qc * 512 : (qc + 1) * 512],
                            vh[:, 2 * p + tl, h, :],
                            exq_prev[p][:, tl * 2048 + qc * 512 : tl * 2048 + (qc + 1) * 512],
                            start=(p == 0 and tl == 0),
                            stop=(p == 7 and tl == 1),
                        )
                if p == 7:
                    hp, hr = h // 2, (h % 2) * 64
                    st = stp.tile([EL, S], FPR, tag="stage", name=f"st_{h}")
                    nc.vector.tensor_copy(st[:], pvt[0][:])
                    nc.gpsimd.dma_start(attnT[hr : hr + 64, hp, :], st[0:E, :])
                    nc.gpsimd.dma_start(sums[h : h + 1, :], st[E : E + 1, :])

            def vproj_unit(tt, vpp, xvt_box):
                c, u = tt // 4, tt % 4
                if u == 0:
                    xvt = xvp.tile([128, DT, 512], BF, tag="xvin", name=f"xvt_{c}")
                    for dt in range(DT):
                        nc.sync.dma_start(
                            xvt[:, dt, :],
                            xv[dt * 128 : (dt + 1) * 128, c * 512 : (c + 1) * 512],
                        )
                    xvt_box[0] = xvt
                ps = vpp.tile([128, HG, E], FP, tag="ppv", name=f"ppv_{tt}")
                for dt in range(DT):
                    nc.tensor.matmul(
                        ps[:],
                        xvt_box[0][:, dt, u * 128 : (u + 1) * 128],
                        wv_sb[:, dt, :],
                        start=(dt == 0),
                        stop=(dt == DT - 1),
                    )
                nc.vector.tensor_copy(vh[:, tt, :, 0:E], ps[:])

            with tc.tile_pool(name="ps_sc", bufs=2, space="PSUM") as scp:
                xvt_box = [None]
                with tc.tile_pool(name="ps_vproj", bufs=2, space="PSUM") as vpp:
                    for tt in range(16):          # head 0 scores + v proj
                        scores_unit(0, tt, scp)
                        vproj_unit(tt, vpp, xvt_box)
                exq_prev, exq_cur = exq_cur, {}
                with tc.tile_pool(name="ps_pv", bufs=1, space="PSUM") as pvp:
                    for h in range(1, HG):        # scores(h) ~ PV(h-1)
                        for tt in range(16):
                            scores_unit(h, tt, scp)
                            # PV pairs as early as possible so exbuf slots
                            # recycle before the ACT that reuses them
                            if tt < 8:
                                pv_pair(h - 1, tt, pvp)
                        exq_prev, exq_cur = exq_cur, {}
                    for p in range(8):            # PV of the last head
                        pv_pair(HG - 1, p, pvp)

            # ---- phase 3: normalize + output projection ------------------
            with nc.allow_low_precision(reason="fp32r recip feeds fp32r matmul"):
                nc.vector.reciprocal(recip[:], sums[:])
            with tc.tile_pool(name="ps_rb", bufs=1, space="PSUM") as prb:
                for j in range(2):
                    rb = prb.tile([128, S], FP, tag="rb")
                    for qc in range(4):
                        nc.tensor.matmul(
                            rb[:, qc * 512 : (qc + 1) * 512],
                            eall[:, j, :],
                            recip[:, qc * 512 : (qc + 1) * 512],
                            start=True,
                            stop=True,
                        )
                    sl = attnT[:, j, :]
                    nc.vector.tensor_mul(sl, sl, rb[:])
            with tc.tile_pool(name="ps_op", bufs=2, space="PSUM") as pop:
                for sti in range(16):
                    op = pop.tile([128, 1024], FP, tag="op")
                    for j in range(2):
                        for hc in range(2):
                            nc.tensor.matmul(
                                op[:, hc * 512 : (hc + 1) * 512],
                                attnT[:, j, sti * 128 : (sti + 1) * 128],
                                wo_sb[j][:, hc * 512 : (hc + 1) * 512],
                                start=(j == 0),
                                stop=(j == 1),
                            )
                    ot = osb.tile([128, 1024], FP, tag="outev")
                    nc.vector.tensor_copy(ot[:], op[:])
                    nc.sync.dma_start(out[sti * 128 : (sti + 1) * 128, :], ot[:])

    nc.compile()
    return nc


def _get_nc():
    global _NC
    if _NC is None:
        _NC = _build()
    return _NC


def _in_maps(q, k, v, Wq, bq, Wk, bk, Wv, bv, Wo, bo):
    import ml_dtypes
    f32 = np.float32
    bf16 = ml_dtypes.bfloat16

    # eall[h, j, m] = 1 iff attnT row m of j-block j belongs to head h
    eall = np.zeros((HG, 2, 128), f32)
    for h in range(HG):
        eall[h, h // 2, (h % 2) * 64 : (h % 2) * 64 + 64] = 1.0

    maps = []
    for c in range(N_CORES):
        b, hg = c // HG, c % HG
        hs = slice(hg * HG, (hg + 1) * HG)  # this core's 4 heads

        wq_h = np.transpose(Wq[hs], (1, 0, 2)).reshape(D, HG * E)
        wk_h = np.transpose(Wk[hs], (1, 0, 2)).reshape(D, HG * E)
        wv_h = np.transpose(Wv[hs], (1, 0, 2)).reshape(D, HG * E)
        qkb_h = np.stack(
            [
                bq[hs][0:2].reshape(-1),
                bq[hs][2:4].reshape(-1),
                bk[hs][0:2].reshape(-1),
                bk[hs][2:4].reshape(-1),
            ],
            axis=1,
        ).astype(f32)
        maps.append(
            {
                "xq": np.ascontiguousarray(q[b].T).astype(bf16),
                "xk": np.ascontiguousarray(k[b].T).astype(bf16),
                "xv": np.ascontiguousarray(v[b].T).astype(bf16),
                "wq": wq_h.astype(bf16),
                "wk": wk_h.astype(bf16),
                "wv": wv_h.astype(bf16),
                "qkb": qkb_h,
                "wo": np.ascontiguousarray(
                    Wo[hg * HG * E : (hg + 1) * HG * E, :], dtype=f32
                ),
                "eall": eall,
            }
        )
    return maps


def _run(inputs, trace=False):
    from concourse.bass_utils import run_bass_kernel_spmd

    nc = _get_nc()
    maps = _in_maps(**inputs)
    res = run_bass_kernel_spmd(nc, maps, list(range(N_CORES)), trace=trace)
    Wo = np.asarray(inputs["Wo"], np.float32)
    bv = np.asarray(inputs["bv"], np.float32)
    bo = np.asarray(inputs["bo"], np.float32)
    bo_eff = bo + bv.reshape(-1) @ Wo   # v bias folded through Wo
    out = np.zeros((B, S, D), np.float32)
    for b in range(B):
        acc = np.zeros((S, D), np.float32)
        for hg in range(HG):
            acc += res.results[b * HG + hg]["out_partial"]
        out[b] = acc + bo_eff[None, :]
    return out, res.exec_time_ns


def kernel(**inputs):
    out, _ = _run(inputs, trace=False)
    return out


def kernel_traced(**inputs):
    return _run(inputs, trace=True)


# revision 10
# speedup vs baseline: 1.1897x; 1.1661x over previous
"""Multi-head attention (B=2, S=2048, D=1024, H=16, E=64) on 8 NeuronCores.

Sharding: core c = (batch b, head-group hg) with b = c // 4, hg = c % 4.
Each core projects q/k/v for its batch into its 4 heads, runs dense
attention over the full sequence, and computes a partial output
projection with its 256 rows of Wo.  The host sums the 4 partials per
batch and adds bo (with the v-bias contribution bv @ Wo folded in).

V2 structure (vs the 350us baseline):
  - scores psum is evicted to SBUF fp16, so EXP runs as 32 ACTs of
    [128, 4096] from SBUF instead of 128 ACTs of [128, 1024] from
    PSUM: ~20% less scalar-engine time and no psum backpressure.
  - per-head processing: scores/EXP for head h interleave with the
    PV accumulation of head h-1 (and with the v projection for h=0),
    so the tensor engine never sits behind the scalar engine.
  - PV accumulates a whole head into one [65, 2048] psum (16 chained
    matmuls, 2048-col moving operand); the 65th row is the softmax
    denominator via a ones-column in vh (memset, not a bias matmul).
  - q/k/v biases: q/k folded into the psum eviction (tensor_scalar_add
    with a per-partition bias vector); v bias folded host-side into bo
    (bv @ Wo).  No K=1 bias matmuls.
  - projections stream x tiles once (dt-outer, all 4 (j, sc2) psums
    open) with 1024-col moving operands.
  - attnT / recip stay fp32r (denominator precision); q/k/attn inputs
    bf16; out f32.
"""

import numpy as np

B, S, D, H, E = 2, 2048, 1024, 16, 64
HG = 4            # heads per core
N_CORES = 8
EL = E + 1        # 65: head block width in vh (values + ones column)
DT = D // 128     # 8 contraction tiles

_NC = None        # cached compiled Bass module


def _build():
    import concourse.bass as bass
    import concourse.mybir as mybir
    import concourse.tile as tile
    from concourse import bacc

    FP = mybir.dt.float32
    FPR = mybir.dt.float32r
    BF = mybir.dt.bfloat16
    F16 = mybir.dt.float16
    EXP = mybir.ActivationFunctionType.Exp

    nc = bacc.Bacc("TRN2", target_bir_lowering=False, debug=False, num_devices=1)

    xq = nc.dram_tensor("xq", [D, S], BF, kind="ExternalInput").ap()
    xk = nc.dram_tensor("xk", [D, S], BF, kind="ExternalInput").ap()
    xv = nc.dram_tensor("xv", [D, S], BF, kind="ExternalInput").ap()
    wq = nc.dram_tensor("wq", [D, HG * E], BF, kind="ExternalInput").ap()
    wk = nc.dram_tensor("wk", [D, HG * E], BF, kind="ExternalInput").ap()
    wv = nc.dram_tensor("wv", [D, HG * E], BF, kind="ExternalInput").ap()
    qkb = nc.dram_tensor("qkb", [128, 4], FP, kind="ExternalInput").ap()
    wo = nc.dram_tensor("wo", [HG * E, D], FPR, kind="ExternalInput").ap()
    eall_d = nc.dram_tensor("eall", [HG, 2, 128], FPR, kind="ExternalInput").ap()
    out = nc.dram_tensor("out_partial", [S, D], BF, kind="ExternalOutput").ap()

    with tile.TileContext(nc) as tc:
        with (
            tc.tile_pool(name="consts", bufs=1) as cpool,
            tc.tile_pool(name="resident", bufs=1) as rpool,
            tc.tile_pool(name="xqk", bufs=3) as xqkp,
            tc.tile_pool(name="xvin", bufs=2) as xvp,
            tc.tile_pool(name="exbuf", bufs=28) as expool,
            tc.tile_pool(name="stage", bufs=1) as stp,
            tc.tile_pool(name="outev", bufs=3) as osb,
        ):
            wq_sb = cpool.tile([128, DT, 256], BF, tag="wq")
            wk_sb = cpool.tile([128, DT, 256], BF, tag="wk")
            wv_sb = cpool.tile([128, DT, 256], BF, tag="wv")
            qkb_sb = cpool.tile([128, 4], FP, tag="qkb")
            eall = cpool.tile([HG, 2, 128], FPR, tag="eall")
            wo_sb = []
            for j in range(2):
                t = cpool.tile([128, D], FPR, tag=f"wo{j}")
                nc.gpsimd.dma_start(t[:], wo[j * 128 : (j + 1) * 128, :])
                wo_sb.append(t)
            for dt in range(DT):
                nc.gpsimd.dma_start(wq_sb[:, dt, :], wq[dt * 128 : (dt + 1) * 128, :])
                nc.gpsimd.dma_start(wk_sb[:, dt, :], wk[dt * 128 : (dt + 1) * 128, :])
                nc.gpsimd.dma_start(wv_sb[:, dt, :], wv[dt * 128 : (dt + 1) * 128, :])
            nc.gpsimd.dma_start(qkb_sb[:], qkb[:])
            nc.gpsimd.dma_start(eall[:], eall_d[:])

            qhT = rpool.tile([128, 2, S], FPR, tag="qhT")
            khT = rpool.tile([128, 2, S], FPR, tag="khT")
            vh = rpool.tile([128, 16, HG, EL], BF, tag="vh")
            attnT = rpool.tile([128, 2, S], FPR, tag="attnT")
            sums = rpool.tile([HG, S], FPR, tag="sums")
            recip = rpool.tile([HG, S], FPR, tag="recip")
            sums_sp = rpool.tile([128, 64], FPR, tag="sums_sp")
            recip_sp = rpool.tile([128, 64], FPR, tag="recip_sp")

            # ones column of vh (softmax denominator accumulates in the
            # PV matmul); projection evictions only write cols 0:64.
            nc.vector.memset(vh[:, :, :, E : E + 1], 1.0)

            # ---- phase 1: q/k projections --------------------------------
            # dt-outer with all 4 (j, sc2) psums open: each x tile is
            # DMA'd once and read by 4 matmuls; w stationary serves 2.
            with tc.tile_pool(name="ps_proj", bufs=4, space="PSUM") as pp:
                for x_dram, w_sb, bcol, dst in (
                    (xq, wq_sb, 0, qhT),
                    (xk, wk_sb, 2, khT),
                ):
                    pss = {}
                    for j in range(2):
                        for sc2 in range(2):
                            pss[j, sc2] = pp.tile(
                                [128, 1024], FP, tag="pp", name=f"pp_{bcol}_{j}_{sc2}"
                            )
                    for dt in range(DT):
                        xt = xqkp.tile([128, S], BF, tag="xqk")
                        nc.sync.dma_start(xt[:], x_dram[dt * 128 : (dt + 1) * 128, :])
                        for j in range(2):
                            for sc2 in range(2):
                                for hc in range(2):
                                    nc.tensor.matmul(
                                        pss[j, sc2][:, hc * 512 : (hc + 1) * 512],
                                        w_sb[:, dt, j * 128 : (j + 1) * 128],
                                        xt[:, sc2 * 1024 + hc * 512 : sc2 * 1024 + (hc + 1) * 512],
                                        start=(dt == 0),
                                        stop=(dt == DT - 1),
                                    )
                    for j in range(2):
                        for sc2 in range(2):
                            nc.vector.tensor_scalar_add(
                                dst[:, j, sc2 * 1024 : (sc2 + 1) * 1024],
                                pss[j, sc2][:],
                                qkb_sb[:, bcol + j : bcol + j + 1],
                            )

            # ---- phase 2: attention (+ v projection interleaved) ---------
            exq_prev = {}   # ex tiles (tt, sc2) of the previous head
            exq_cur = {}
            pvt = [None]    # open PV psum of the previous head
            cur_sc = [None]

            def scores_unit(h, tt, scp):
                hp, hr = h // 2, (h % 2) * 64
                for sc2 in range(2):
                    ps = scp.tile([128, 1024], FP, tag="scps", name=f"scps_{h}_{tt}_{sc2}")
                    for hc in range(2):
                        nc.tensor.matmul(
                            ps[:, hc * 512 : (hc + 1) * 512],
                            khT[hr : hr + 64, hp, tt * 128 : (tt + 1) * 128],
                            qhT[hr : hr + 64, hp, sc2 * 1024 + hc * 512 : sc2 * 1024 + (hc + 1) * 512],
                            start=True,
                            stop=True,
                        )
                    exb = expool.tile([128, 1024], BF, tag="exb", name=f"exb_{h}_{tt}_{sc2}")
                    nc.scalar.activation(exb[:], ps[:], EXP, scale=0.125)
                    exq_cur[tt, sc2] = exb

            def pv_pair(h, p, pvp):
                if p == 0:
                    pvt[0] = pvp.tile([EL, S], FP, tag="pv", name=f"pv_{h}")
                for tl in range(2):
                    tt = 2 * p + tl
                    for qc in range(4):
                        sc2, hc = qc // 2, qc % 2
                        nc.tensor.matmul(
                            pvt[0][:, qc * 512 : (qc + 1) * 512],
                            vh[:, tt, h, :],
                            exq_prev[tt, sc2][:, hc * 512 : (hc + 1) * 512],
                            start=(p == 0 and tl == 0),
                            stop=(p == 7 and tl == 1),
                        )
                if p == 7:
                    hp, hr = h // 2, (h % 2) * 64
                    st = stp.tile([EL, S], FPR, tag="stage", name=f"st_{h}")
                    nc.vector.tensor_copy(st[:], pvt[0][:])
                    nc.gpsimd.dma_start(attnT[hr : hr + 64, hp, :], st[0:E, :])
                    # spread the denominator over 16 partitions so the
                    # reciprocal runs wide, then gather back to [1, 2048]
                    nc.gpsimd.dma_start(sums_sp[h * 32 : (h + 1) * 32, :], st[E : E + 1, :])
                    with nc.allow_low_precision(reason="fp32r recip, fp32r rb matmul"):
                        nc.vector.reciprocal(
                            recip_sp[h * 32 : (h + 1) * 32, :],
                            sums_sp[h * 32 : (h + 1) * 32, :],
                        )
                    nc.gpsimd.dma_start(recip[h : h + 1, :], recip_sp[h * 32 : (h + 1) * 32, :])

            def vproj_unit(tt, vpp, xvt_box):
                c, u = tt // 4, tt % 4
                if u == 0:
                    xvt = xvp.tile([128, DT, 512], BF, tag="xvin", name=f"xvt_{c}")
                    for dt in range(DT):
                        nc.sync.dma_start(
                            xvt[:, dt, :],
                            xv[dt * 128 : (dt + 1) * 128, c * 512 : (c + 1) * 512],
                        )
                    xvt_box[0] = xvt
                ps = vpp.tile([128, HG, E], FP, tag="ppv", name=f"ppv_{tt}")
                for dt in range(DT):
                    nc.tensor.matmul(
                        ps[:],
                        xvt_box[0][:, dt, u * 128 : (u + 1) * 128],
                        wv_sb[:, dt, :],
                        start=(dt == 0),
                        stop=(dt == DT - 1),
                    )
                nc.vector.tensor_copy(vh[:, tt, :, 0:E], ps[:])

            with tc.tile_pool(name="ps_sc", bufs=2, space="PSUM") as scp:
                xvt_box = [None]
                with tc.tile_pool(name="ps_vproj", bufs=2, space="PSUM") as vpp:
                    for tt in range(16):          # head 0 scores + v proj
                        scores_unit(0, tt, scp)
                        vproj_unit(tt, vpp, xvt_box)
                exq_prev, exq_cur = exq_cur, {}
                with tc.tile_pool(name="ps_pv", bufs=1, space="PSUM") as pvp:
                    for h in range(1, HG):        # scores(h) ~ PV(h-1)
                        for tt in range(16):
                            scores_unit(h, tt, scp)
                            # PV pairs as early as possible so exbuf slots
                            # recycle before the ACT that reuses them
                            if tt < 8:
                                pv_pair(h - 1, tt, pvp)
                        exq_prev, exq_cur = exq_cur, {}
                    for p in range(8):            # PV of the last head
                        pv_pair(HG - 1, p, pvp)

            # ---- phase 3: normalize + output projection ------------------
            with tc.tile_pool(name="ps_rb", bufs=1, space="PSUM") as prb:
                for j in range(2):
                    rb = prb.tile([128, S], FP, tag="rb")
                    for qc in range(4):
                        nc.tensor.matmul(
                            rb[:, qc * 512 : (qc + 1) * 512],
                            eall[:, j, :],
                            recip[:, qc * 512 : (qc + 1) * 512],
                            start=True,
                            stop=True,
                        )
                    sl = attnT[:, j, :]
                    nc.vector.tensor_mul(sl, sl, rb[:])
            with tc.tile_pool(name="ps_op", bufs=2, space="PSUM") as pop:
                for sti in range(16):
                    op = pop.tile([128, 1024], FP, tag="op")
                    for j in range(2):
                        for hc in range(2):
                            nc.tensor.matmul(
                                op[:, hc * 512 : (hc + 1) * 512],
                                attnT[:, j, sti * 128 : (sti + 1) * 128],
                                wo_sb[j][:, hc * 512 : (hc + 1) * 512],
                                start=(j == 0),
                                stop=(j == 1),
                            )
                    ot = osb.tile([128, 1024], BF, tag="outev")
                    if sti % 2 == 0:
                        nc.vector.tensor_copy(ot[:], op[:])
                    else:
                        nc.scalar.copy(ot[:], op[:])
                    nc.sync.dma_start(out[sti * 128 : (sti + 1) * 128, :], ot[:])

    nc.compile()
    return nc


def _get_nc():
    global _NC
    if _NC is None:
        _NC = _build()
    return _NC


def _in_maps(q, k, v, Wq, bq, Wk, bk, Wv, bv, Wo, bo):
    import ml_dtypes
    f32 = np.float32
    bf16 = ml_dtypes.bfloat16

    # eall[h, j, m] = 1 iff attnT row m of j-block j belongs to head h
    eall = np.zeros((HG, 2, 128), f32)
    for h in range(HG):
        eall[h, h // 2, (h % 2) * 64 : (h % 2) * 64 + 64] = 1.0

    maps = []
    for c in range(N_CORES):
        b, hg = c // HG, c % HG
        hs = slice(hg * HG, (hg + 1) * HG)  # this core's 4 heads

        wq_h = np.transpose(Wq[hs], (1, 0, 2)).reshape(D, HG * E)
        wk_h = np.transpose(Wk[hs], (1, 0, 2)).reshape(D, HG * E)
        wv_h = np.transpose(Wv[hs], (1, 0, 2)).reshape(D, HG * E)
        qkb_h = np.stack(
            [
                bq[hs][0:2].reshape(-1),
                bq[hs][2:4].reshape(-1),
                bk[hs][0:2].reshape(-1),
                bk[hs][2:4].reshape(-1),
            ],
            axis=1,
        ).astype(f32)
        maps.append(
            {
                "xq": np.ascontiguousarray(q[b].T).astype(bf16),
                "xk": np.ascontiguousarray(k[b].T).astype(bf16),
                "xv": np.ascontiguousarray(v[b].T).astype(bf16),
                "wq": wq_h.astype(bf16),
                "wk": wk_h.astype(bf16),
                "wv": wv_h.astype(bf16),
                "qkb": qkb_h,
                "wo": np.ascontiguousarray(
                    Wo[hg * HG * E : (hg + 1) * HG * E, :], dtype=f32
                ),
                "eall": eall,
            }
        )
    return maps


def _run(inputs, trace=False):
    from concourse.bass_utils import run_bass_kernel_spmd

    nc = _get_nc()
    maps = _in_maps(**inputs)
    res = run_bass_kernel_spmd(nc, maps, list(range(N_CORES)), trace=trace)
    Wo = np.asarray(inputs["Wo"], np.float32)
    bv = np.asarray(inputs["bv"], np.float32)
    bo = np.asarray(inputs["bo"], np.float32)
    bo_eff = bo + bv.reshape(-1) @ Wo   # v bias folded through Wo
    out = np.zeros((B, S, D), np.float32)
    for b in range(B):
        acc = np.zeros((S, D), np.float32)
        for hg in range(HG):
            acc += res.results[b * HG + hg]["out_partial"].astype(np.float32)
        out[b] = acc + bo_eff[None, :]
    return out, res.exec_time_ns


def kernel(**inputs):
    out, _ = _run(inputs, trace=False)
    return out


def kernel_traced(**inputs):
    return _run(inputs, trace=True)


# revision 13
# speedup vs baseline: 1.2820x; 1.0775x over previous
"""Multi-head attention (B=2, S=2048, D=1024, H=16, E=64) on 8 NeuronCores.

Sharding: core c = (batch b, head-group hg) with b = c // 4, hg = c % 4.
Each core projects q/k/v for its batch into its 4 heads, runs dense
attention over the full sequence, and computes a partial output
projection with its 256 rows of Wo.  The host sums the 4 partials per
batch and adds bo (with the v-bias contribution bv @ Wo folded in).

V2 structure (vs the 350us baseline):
  - scores psum is evicted to SBUF fp16, so EXP runs as 32 ACTs of
    [128, 4096] from SBUF instead of 128 ACTs of [128, 1024] from
    PSUM: ~20% less scalar-engine time and no psum backpressure.
  - per-head processing: scores/EXP for head h interleave with the
    PV accumulation of head h-1 (and with the v projection for h=0),
    so the tensor engine never sits behind the scalar engine.
  - PV accumulates a whole head into one [65, 2048] psum (16 chained
    matmuls, 2048-col moving operand); the 65th row is the softmax
    denominator via a ones-column in vh (memset, not a bias matmul).
  - q/k/v biases: q/k folded into the psum eviction (tensor_scalar_add
    with a per-partition bias vector); v bias folded host-side into bo
    (bv @ Wo).  No K=1 bias matmuls.
  - projections stream x tiles once (dt-outer, all 4 (j, sc2) psums
    open) with 1024-col moving operands.
  - attnT / recip stay fp32r (denominator precision); q/k/attn inputs
    bf16; out f32.
"""

import numpy as np

B, S, D, H, E = 2, 2048, 1024, 16, 64
HG = 4            # heads per core
N_CORES = 8
EL = E + 1        # 65: head block width in vh (values + ones column)
DT = D // 128     # 8 contraction tiles

_NC = None        # cached compiled Bass module


def _build():
    import concourse.bass as bass
    import concourse.mybir as mybir
    import concourse.tile as tile
    from concourse import bacc

    FP = mybir.dt.float32
    FPR = mybir.dt.float32r
    BF = mybir.dt.bfloat16
    F16 = mybir.dt.float16
    EXP = mybir.ActivationFunctionType.Exp

    nc = bacc.Bacc("TRN2", target_bir_lowering=False, debug=False, num_devices=1)

    xq = nc.dram_tensor("xq", [D, S], BF, kind="ExternalInput").ap()
    xk = nc.dram_tensor("xk", [D, S], BF, kind="ExternalInput").ap()
    xv = nc.dram_tensor("xv", [D, S], BF, kind="ExternalInput").ap()
    wq = nc.dram_tensor("wq", [D, HG * E], BF, kind="ExternalInput").ap()
    wk = nc.dram_tensor("wk", [D, HG * E], BF, kind="ExternalInput").ap()
    wv = nc.dram_tensor("wv", [D, HG * E], BF, kind="ExternalInput").ap()
    qkb = nc.dram_tensor("qkb", [128, 4], FP, kind="ExternalInput").ap()
    wo = nc.dram_tensor("wo", [HG * E, D], FPR, kind="ExternalInput").ap()
    eall_d = nc.dram_tensor("eall", [HG, 2, 128], FPR, kind="ExternalInput").ap()
    out = nc.dram_tensor("out_partial", [S, D], BF, kind="ExternalOutput").ap()

    with tile.TileContext(nc) as tc:
        with (
            tc.tile_pool(name="consts", bufs=1) as cpool,
            tc.tile_pool(name="resident", bufs=1) as rpool,
            tc.tile_pool(name="xqk", bufs=3) as xqkp,
            tc.tile_pool(name="xvin", bufs=2) as xvp,
            tc.tile_pool(name="exbuf", bufs=28) as expool,
            tc.tile_pool(name="stage", bufs=1) as stp,
            tc.tile_pool(name="outev", bufs=3) as osb,
        ):
            wq_sb = cpool.tile([128, DT, 256], BF, tag="wq")
            wk_sb = cpool.tile([128, DT, 256], BF, tag="wk")
            wv_sb = cpool.tile([128, DT, 256], BF, tag="wv")
            qkb_sb = cpool.tile([128, 4], FP, tag="qkb")
            eall = cpool.tile([HG, 2, 128], FPR, tag="eall")
            wo_sb = [cpool.tile([128, D], FPR, tag=f"wo{j}", name=f"wo_sb{j}") for j in range(2)]
            for dt in range(DT):
                nc.gpsimd.dma_start(wq_sb[:, dt, :], wq[dt * 128 : (dt + 1) * 128, :])
                nc.gpsimd.dma_start(wk_sb[:, dt, :], wk[dt * 128 : (dt + 1) * 128, :])
            nc.gpsimd.dma_start(qkb_sb[:], qkb[:])
            for dt in range(DT):
                nc.gpsimd.dma_start(wv_sb[:, dt, :], wv[dt * 128 : (dt + 1) * 128, :])
            nc.gpsimd.dma_start(eall[:], eall_d[:])
            for j in range(2):
                nc.gpsimd.dma_start(wo_sb[j][:], wo[j * 128 : (j + 1) * 128, :])

            qhT = rpool.tile([128, 2, S], BF, tag="qhT")
            khT = rpool.tile([128, 2, S], BF, tag="khT")
            qhTs = rpool.tile([128, 2, S], BF, tag="qhTs")   # row-halves swapped
            khTs = rpool.tile([128, 2, S], BF, tag="khTs")
            vh = rpool.tile([128, 16, HG, EL], BF, tag="vh")
            attnT = rpool.tile([128, 2, S], FPR, tag="attnT")
            sums = rpool.tile([HG, S], FPR, tag="sums")
            recip = rpool.tile([HG, S], FPR, tag="recip")
            sums_sp = rpool.tile([128, 64], FPR, tag="sums_sp")
            recip_sp = rpool.tile([128, 64], FPR, tag="recip_sp")

            # ones column of vh (softmax denominator accumulates in the
            # PV matmul); projection evictions only write cols 0:64.
            nc.vector.memset(vh[:, :, :, E : E + 1], 1.0)

            # ---- phase 1: q/k projections --------------------------------
            # dt-outer with all 4 (j, sc2) psums open: each x tile is
            # DMA'd once and read by 4 matmuls; w stationary serves 2.
            with tc.tile_pool(name="ps_proj", bufs=4, space="PSUM") as pp:
                for x_dram, w_sb, bcol, dst in (
                    (xq, wq_sb, 0, qhT),
                    (xk, wk_sb, 2, khT),
                ):
                    pss = {}
                    for j in range(2):
                        for sc2 in range(2):
                            pss[j, sc2] = pp.tile(
                                [128, 1024], FP, tag="pp", name=f"pp_{bcol}_{j}_{sc2}"
                            )
                    for dt in range(DT):
                        xt = xqkp.tile([128, S], BF, tag="xqk")
                        nc.sync.dma_start(xt[:], x_dram[dt * 128 : (dt + 1) * 128, :])
                        for j in range(2):
                            for sc2 in range(2):
                                for hc in range(2):
                                    nc.tensor.matmul(
                                        pss[j, sc2][:, hc * 512 : (hc + 1) * 512],
                                        w_sb[:, dt, j * 128 : (j + 1) * 128],
                                        xt[:, sc2 * 1024 + hc * 512 : sc2 * 1024 + (hc + 1) * 512],
                                        start=(dt == 0),
                                        stop=(dt == DT - 1),
                                    )
                    for j in range(2):
                        for sc2 in range(2):
                            nc.vector.tensor_scalar_add(
                                dst[:, j, sc2 * 1024 : (sc2 + 1) * 1024],
                                pss[j, sc2][:],
                                qkb_sb[:, bcol + j : bcol + j + 1],
                            )
                    # row-swapped duplicate: lets scores alternate PE row
                    # quadrants per key block so LDWEIGHTS overlaps matmuls
                    dsts = qhTs if dst is qhT else khTs
                    nc.gpsimd.dma_start(dsts[0:64, :, :], dst[64:128, :, :])
                    nc.gpsimd.dma_start(dsts[64:128, :, :], dst[0:64, :, :])

            # ---- phase 2: attention (+ v projection interleaved) ---------
            exq_prev = {}   # ex tiles (tt, sc2) of the previous head
            exq_cur = {}
            pvt = [None]    # open PV psum of the previous head
            cur_sc = [None]

            def scores_unit(h, tt, scp):
                hp = h // 2
                if tt % 2 == 0:
                    kt, qt, hr = khT, qhT, (h % 2) * 64
                else:
                    kt, qt, hr = khTs, qhTs, (1 - h % 2) * 64
                for sc2 in range(2):
                    ps = scp.tile([128, 1024], FP, tag="scps", name=f"scps_{h}_{tt}_{sc2}")
                    for hc in range(2):
                        nc.tensor.matmul(
                            ps[:, hc * 512 : (hc + 1) * 512],
                            kt[hr : hr + 64, hp, tt * 128 : (tt + 1) * 128],
                            qt[hr : hr + 64, hp, sc2 * 1024 + hc * 512 : sc2 * 1024 + (hc + 1) * 512],
                            start=True,
                            stop=True,
                        )
                    exb = expool.tile([128, 1024], BF, tag="exb", name=f"exb_{h}_{tt}_{sc2}")
                    nc.scalar.activation(exb[:], ps[:], EXP, scale=0.125)
                    exq_cur[tt, sc2] = exb

            def pv_pair(h, p, pvp):
                if p == 0:
                    pvt[0] = pvp.tile([EL, S], FP, tag="pv", name=f"pv_{h}")
                for tl in range(2):
                    tt = 2 * p + tl
                    for qc in range(4):
                        sc2, hc = qc // 2, qc % 2
                        nc.tensor.matmul(
                            pvt[0][:, qc * 512 : (qc + 1) * 512],
                            vh[:, tt, h, :],
                            exq_prev[tt, sc2][:, hc * 512 : (hc + 1) * 512],
                            start=(p == 0 and tl == 0),
                            stop=(p == 7 and tl == 1),
                        )
                if p == 7:
                    hp, hr = h // 2, (h % 2) * 64
                    st = stp.tile([EL, S], FPR, tag="stage", name=f"st_{h}")
                    nc.vector.tensor_copy(st[:], pvt[0][:])
                    nc.gpsimd.dma_start(attnT[hr : hr + 64, hp, :], st[0:E, :])
                    # spread the denominator over 16 partitions so the
                    # reciprocal runs wide, then gather back to [1, 2048]
                    nc.gpsimd.dma_start(sums_sp[h * 32 : (h + 1) * 32, :], st[E : E + 1, :])
                    with nc.allow_low_precision(reason="fp32r recip, fp32r rb matmul"):
                        nc.vector.reciprocal(
                            recip_sp[h * 32 : (h + 1) * 32, :],
                            sums_sp[h * 32 : (h + 1) * 32, :],
                        )
                    nc.gpsimd.dma_start(recip[h : h + 1, :], recip_sp[h * 32 : (h + 1) * 32, :])

            def vproj_unit(tt, vpp, xvt_box):
                c, u = tt // 4, tt % 4
                if u == 0:
                    xvt = xvp.tile([128, DT, 512], BF, tag="xvin", name=f"xvt_{c}")
                    for dt in range(DT):
                        nc.sync.dma_start(
                            xvt[:, dt, :],
                            xv[dt * 128 : (dt + 1) * 128, c * 512 : (c + 1) * 512],
                        )
                    xvt_box[0] = xvt
                ps = vpp.tile([128, HG, E], FP, tag="ppv", name=f"ppv_{tt}")
                for dt in range(DT):
                    nc.tensor.matmul(
                        ps[:],
                        xvt_box[0][:, dt, u * 128 : (u + 1) * 128],
                        wv_sb[:, dt, :],
                        start=(dt == 0),
                        stop=(dt == DT - 1),
                    )
                nc.vector.tensor_copy(vh[:, tt, :, 0:E], ps[:])

            with tc.tile_pool(name="ps_sc", bufs=2, space="PSUM") as scp:
                xvt_box = [None]
                with tc.tile_pool(name="ps_vproj", bufs=2, space="PSUM") as vpp:
                    for tt in range(16):          # head 0 scores + v proj
                        scores_unit(0, tt, scp)
                        vproj_unit(tt, vpp, xvt_box)
                exq_prev, exq_cur = exq_cur, {}
                def norm_j(j):
                    # normalize via scores-pool psum tiles (free by now)
                    for half in range(2):
                        rb = scp.tile([128, 1024], FP, tag="scps", name=f"rb_{j}_{half}")
                        for qc in range(2):
                            nc.tensor.matmul(
                                rb[:, qc * 512 : (qc + 1) * 512],
                                eall[:, j, :],
                                recip[:, half * 1024 + qc * 512 : half * 1024 + (qc + 1) * 512],
                                start=True,
                                stop=True,
                            )
                        sl = attnT[:, j, half * 1024 : (half + 1) * 1024]
                        nc.vector.tensor_mul(sl, sl, rb[:])

                with tc.tile_pool(name="ps_pv", bufs=1, space="PSUM") as pvp:
                    for h in range(1, HG):        # scores(h) ~ PV(h-1)
                        for tt in range(16):
                            scores_unit(h, tt, scp)
                            # PV pairs as early as possible so exbuf slots
                            # recycle before the ACT that reuses them
                            if tt < 8:
                                pv_pair(h - 1, tt, pvp)
                        exq_prev, exq_cur = exq_cur, {}
                    for p in range(8):            # PV of the last head
                        pv_pair(HG - 1, p, pvp)
                        if p == 3:
                            norm_j(0)             # heads 0/1 already final
                norm_j(1)
            with tc.tile_pool(name="ps_op", bufs=2, space="PSUM") as pop:
                for sti in range(16):
                    op = pop.tile([128, 1024], FP, tag="op")
                    for j in range(2):
                        for hc in range(2):
                            nc.tensor.matmul(
                                op[:, hc * 512 : (hc + 1) * 512],
                                attnT[:, j, sti * 128 : (sti + 1) * 128],
                                wo_sb[j][:, hc * 512 : (hc + 1) * 512],
                                start=(j == 0),
                                stop=(j == 1),
                            )
                    ot = osb.tile([128, 1024], BF, tag="outev")
                    if sti % 2 == 0:
                        nc.vector.tensor_copy(ot[:], op[:])
                    else:
                        nc.scalar.copy(ot[:], op[:])
                    nc.sync.dma_start(out[sti * 128 : (sti + 1) * 128, :], ot[:])

    nc.compile()
    return nc


def _get_nc():
    global _NC
    if _NC is None:
        _NC = _build()
    return _NC


def _in_maps(q, k, v, Wq, bq, Wk, bk, Wv, bv, Wo, bo):
    import ml_dtypes
    f32 = np.float32
    bf16 = ml_dtypes.bfloat16

    # eall[h, j, m] = 1 iff attnT row m of j-block j belongs to head h
    eall = np.zeros((HG, 2, 128), f32)
    for h in range(HG):
        eall[h, h // 2, (h % 2) * 64 : (h % 2) * 64 + 64] = 1.0

    maps = []
    for c in range(N_CORES):
        b, hg = c // HG, c % HG
        hs = slice(hg * HG, (hg + 1) * HG)  # this core's 4 heads

        wq_h = np.transpose(Wq[hs], (1, 0, 2)).reshape(D, HG * E)
        wk_h = np.transpose(Wk[hs], (1, 0, 2)).reshape(D, HG * E)
        wv_h = np.transpose(Wv[hs], (1, 0, 2)).reshape(D, HG * E)
        qkb_h = np.stack(
            [
                bq[hs][0:2].reshape(-1),
                bq[hs][2:4].reshape(-1),
                bk[hs][0:2].reshape(-1),
                bk[hs][2:4].reshape(-1),
            ],
            axis=1,
        ).astype(f32)
        maps.append(
            {
                "xq": np.ascontiguousarray(q[b].T).astype(bf16),
                "xk": np.ascontiguousarray(k[b].T).astype(bf16),
                "xv": np.ascontiguousarray(v[b].T).astype(bf16),
                "wq": wq_h.astype(bf16),
                "wk": wk_h.astype(bf16),
                "wv": wv_h.astype(bf16),
                "qkb": qkb_h,
                "wo": np.ascontiguousarray(
                    Wo[hg * HG * E : (hg + 1) * HG * E, :], dtype=f32
                ),
                "eall": eall,
            }
        )
    return maps


def _run(inputs, trace=False):
    from concourse.bass_utils import run_bass_kernel_spmd

    nc = _get_nc()
    maps = _in_maps(**inputs)
    res = run_bass_kernel_spmd(nc, maps, list(range(N_CORES)), trace=trace)
    Wo = np.asarray(inputs["Wo"], np.float32)
    bv = np.asarray(inputs["bv"], np.float32)
    bo = np.asarray(inputs["bo"], np.float32)
    bo_eff = bo + bv.reshape(-1) @ Wo   # v bias folded through Wo
    out = np.zeros((B, S, D), np.float32)
    for b in range(B):
        acc = np.zeros((S, D), np.float32)
        for hg in range(HG):
            acc += res.results[b * HG + hg]["out_partial"].astype(np.float32)
        out[b] = acc + bo_eff[None, :]
    return out, res.exec_time_ns


def kernel(**inputs):
    out, _ = _run(inputs, trace=False)
    return out


def kernel_traced(**inputs):
    return _run(inputs, trace=True)
